# revision 22
# baseline (speedup 1.0000x reference)
"""Causal self-attention (B=4, S=2048, D=1024, H=16) on 8 NeuronCores.

Sharding (per spec hint): data-parallel over B (4 batches) x tensor-parallel
over heads (2 groups of 8 heads) -> 8 shards, one per core. Each core:
  - QKV^T projection for its batch + head group (Megatron column split)
  - causal attention for its 8 heads, computed transposed: S^T[sk, sq] tiles
    so the softmax denominator (ones-column trick) and P@V both have the key
    dim on partitions
  - output projection with the Megatron row split of W_proj -> partial out^T
Host sums the two partials per batch and adds the bias terms.

Numerics: bf16 matmul operands (x, W, Q^T, K^T, V, P, y) with fp32 PSUM
accumulation; softmax (exp, denominators, reciprocal) in fp32. The softmax
max-subtraction is skipped: scores = QK/8 for these inputs are ~N(0,1), so
exp() cannot overflow and the result is mathematically identical.

Head pairs are packed: S^T for two heads runs as concurrent row-tiled
matmuls (K=64 each at tile_position (0,0)/(64,0)) into one 2-bank PSUM tile,
so exp covers both heads in ONE activation instruction (the per-instruction
~352-cycle ACT overhead is the attention bottleneck otherwise). Diagonal
chunks restrict the query-column range (c >= 128*d) so ~38% of the wasted
upper-triangle exp/matmul work is skipped; only the 128-wide boundary strip
needs the triangular mask.

b_attn's Q/K parts bias the scores nonlinearly -> added on-device (free,
during the PSUM->SBUF copies of Q^T/K^T). b_attn's V part and b_proj enter
the output linearly -> added host-side (y@W_proj + (b_v@W_proj + b_proj)).
"""

import numpy as np

B, S, D, H = 4, 2048, 1024, 16
HD = 64          # head dim
HG = H // 2      # heads per core (group)
NPAIR = HG // 2  # head pairs per core
DL = HG * HD     # local d (per group) = 512
SQT = 512        # query-tile width
SKC = 128        # key-chunk width

_CACHE = {}


def _build(seq=S, n_pairs=NPAIR):
    import concourse.bass as bass
    import concourse.tile as tile
    from concourse import bacc, mybir

    f32 = mybir.dt.float32
    bf16 = mybir.dt.bfloat16
    AF = mybir.ActivationFunctionType

    dl = n_pairs * 2 * HD            # local d
    n_st = seq // SQT                # s-tiles
    n_sk = seq // SKC                # sk chunks
    n_dc = D // 128                  # contraction chunks over D
    n_oc = D // 128                  # output-projection d_out chunks
    n_jc = 2 * n_pairs               # q + k row-chunks of 128
    n_sc = SQT // SKC                # sk chunks per s-tile

    nc = bacc.Bacc("TRN2", target_bir_lowering=False, debug=False)

    xT_d = nc.dram_tensor("xT", [n_st, n_dc, 128, SQT], bf16, kind="ExternalInput")
    wqk_d = nc.dram_tensor("wqk", [n_dc, 128, 2 * dl], bf16, kind="ExternalInput")
    wv_d = nc.dram_tensor("wv", [n_dc, 128, dl], bf16, kind="ExternalInput")
    wpj_d = nc.dram_tensor("wpj", [dl, D], bf16, kind="ExternalInput")
    bqk_d = nc.dram_tensor("bqk", [2 * dl], f32, kind="ExternalInput")
    outT_d = nc.dram_tensor(
        "outT", [n_st, n_oc, 128, SQT], f32, kind="ExternalOutput"
    )

    wpj_r = wpj_d[:, :].rearrange("(c p) o -> p c o", p=128)
    bqk_r = bqk_d.ap().rearrange("(c p) -> p c", p=128)

    with tile.TileContext(nc) as tc:
        with (
            tc.tile_pool(name="persist", bufs=1) as persist,
            tc.tile_pool(name="xt", bufs=3) as xt_pool,
            tc.tile_pool(name="qt", bufs=2) as qt_pool,
            tc.tile_pool(name="yp", bufs=2) as y_pool,
            tc.tile_pool(name="pp", bufs=10) as p_pool,
            tc.tile_pool(name="rp", bufs=4) as r_pool,
            tc.tile_pool(name="Rp", bufs=4) as R_pool,
            tc.tile_pool(name="so", bufs=6) as so_pool,
            tc.tile_pool(name="pm", bufs=2, space="PSUM") as pm,
            tc.tile_pool(name="st", bufs=2, space="PSUM") as st_pool,
            tc.tile_pool(name="pv", bufs=2, space="PSUM") as pv_pool,
        ):
            # ---- persistent loads (split per d-chunk for early start) --
            wqk_sb = persist.tile([128, n_dc, 2 * dl], bf16, tag="wqk")
            wv_sb = persist.tile([128, n_dc, dl], bf16, tag="wv")
            wpj_sb = persist.tile([128, n_pairs, D], bf16, tag="wpj")
            bqk_sb = persist.tile([128, n_jc], f32, tag="bqk")
            nc.sync.dma_start(bqk_sb[:], bqk_r)

            kt = persist.tile([128, n_pairs, seq], bf16, tag="kt")
            v = persist.tile([128, n_sk, 2 * n_pairs, HD + 1], bf16, tag="v")

            # triangular boundary mask: M[p, c] = 1 iff c >= p
            mask0 = persist.tile([128, SKC], bf16, tag="mask0")
            nc.gpsimd.memset(mask0[:], 1.0)
            nc.gpsimd.affine_select(
                out=mask0[:], in_=mask0[:],
                compare_op=mybir.AluOpType.is_ge,
                fill=0.0, base=0,
                pattern=[[1, SKC]], channel_multiplier=-1,
            )

            y_hist = {}

            def _emit_proj(tp):
                qp = SQT * tp
                strips = y_hist.pop(tp)
                for oc in range(n_oc):
                    ps = pm.tile([128, SQT], f32, tag="pm")
                    for i in range(n_pairs):
                        nc.tensor.matmul(
                            ps[:],
                            wpj_sb[:, i, 128 * oc : 128 * oc + 128],
                            strips[i][:],
                            start=(i == 0), stop=(i == n_pairs - 1),
                        )
                    ot = so_pool.tile([128, SQT], f32, tag="ot")
                    nc.scalar.copy(ot[:], ps[:])
                    nc.sync.dma_start(outT_d[tp, oc, :, :], ot[:])

            for t in range(n_st):
                q0 = SQT * t
                # ==== QKV^T projection for s-tile t ====================
                xt = xt_pool.tile([128, n_dc, SQT], bf16, tag="xt")
                if t == 0:
                    # interleave so MM(dc) inputs arrive in accumulation order
                    for dc in range(n_dc):
                        nc.sync.dma_start(xt[:, dc, :], xT_d[t, dc, :, :])
                        nc.sync.dma_start(wqk_sb[:, dc, :], wqk_d[dc, :, :])
                    for dc in range(n_dc):
                        nc.sync.dma_start(wv_sb[:, dc, :], wv_d[dc, :, :])
                    for i in range(n_pairs):
                        nc.sync.dma_start(wpj_sb[:, i, :], wpj_r[:, i, :])
                else:
                    for dc in range(n_dc):
                        nc.sync.dma_start(xt[:, dc, :], xT_d[t, dc, :, :])
                qt = qt_pool.tile([128, n_pairs, SQT], bf16, tag="qt")
                for jc in range(n_jc):
                    ps = pm.tile([128, SQT], f32, tag="pm")
                    for dc in range(n_dc):
                        nc.tensor.matmul(
                            ps[:],
                            wqk_sb[:, dc, 128 * jc : 128 * jc + 128],
                            xt[:, dc, :],
                            start=(dc == 0), stop=(dc == n_dc - 1),
                        )
                    dst = (
                        qt[:, jc, :] if jc < n_pairs
                        else kt[:, jc - n_pairs, q0 : q0 + SQT]
                    )
                    nc.vector.tensor_scalar_add(
                        dst, ps[:], bqk_sb[:, jc : jc + 1]
                    )
                for sc in range(n_sc):
                    k = n_sc * t + sc
                    ps = pm.tile([128, dl], f32, tag="pm")
                    for dc in range(n_dc):
                        nc.tensor.matmul(
                            ps[:],
                            xt[:, dc, 128 * sc : 128 * sc + 128],
                            wv_sb[:, dc, :],
                            start=(dc == 0), stop=(dc == n_dc - 1),
                        )
                    nc.scalar.copy(
                        v[:, k, :, 0:HD],
                        ps[:].rearrange("p (h c) -> p h c", h=2 * n_pairs),
                    )
                    nc.vector.memset(v[:, k, :, HD : HD + 1], 1.0)

                # ==== causal attention for sq-tile t ===================
                nch = n_sc * (t + 1)  # sk chunks 0..nch-1
                y_strips = {}
                for i in range(n_pairs):
                    oA = pv_pool.tile([HD + 1, SQT], f32, tag="pv")
                    oB = pv_pool.tile([HD + 1, SQT], f32, tag="pv")
                    for k in range(nch):
                        dgi = k - n_sc * t   # >=0: diagonal chunk index
                        c0 = max(0, 128 * dgi)  # first valid query column
                        stk = st_pool.tile([128, 2, SQT], f32, tag="st")
                        nc.tensor.matmul(
                            stk[:, 0, c0:SQT],
                            kt[0:64, i, 128 * k : 128 * k + 128],
                            qt[0:64, i, c0:SQT],
                            start=True, stop=True, tile_position=(0, 0),
                        )
                        nc.tensor.matmul(
                            stk[:, 1, c0:SQT],
                            kt[64:128, i, 128 * k : 128 * k + 128],
                            qt[64:128, i, c0:SQT],
                            start=True, stop=True, tile_position=(64, 0),
                        )
                        pk = p_pool.tile([128, 2, SQT], bf16, tag="p")
                        nc.scalar.activation(
                            pk[:, :, c0:SQT], stk[:, :, c0:SQT],
                            AF.Exp, scale=0.125,
                        )
                        if dgi >= 0:
                            # triangular boundary strip [c0, c0+128)
                            for h in range(2):
                                nc.vector.tensor_mul(
                                    pk[:, h, c0 : c0 + SKC],
                                    pk[:, h, c0 : c0 + SKC],
                                    mask0[:],
                                )
                        for h, o in ((0, oA), (1, oB)):
                            nc.tensor.matmul(
                                o[:, c0:SQT],
                                v[:, k, 2 * i + h, :],
                                pk[:, h, c0:SQT],
                                start=(k == 0), stop=(k == nch - 1),
                                skip_group_check=True,
                            )
                    # copy l rows PSUM->SBUF (the approx-recip bit trick is
                    # PSUM-incompatible), then fast reciprocal on SBUF
                    lA = r_pool.tile([1, SQT], f32, tag="lc")
                    lB = r_pool.tile([1, SQT], f32, tag="lc")
                    nc.vector.tensor_copy(lA[:], oA[HD : HD + 1, :])
                    nc.vector.tensor_copy(lB[:], oB[HD : HD + 1, :])
                    recA = r_pool.tile([1, SQT], f32, tag="rec")
                    recB = r_pool.tile([1, SQT], f32, tag="rec")
                    nc.vector.reciprocal_approx_fast(recA[:], lA[:])
                    nc.vector.reciprocal_approx_fast(recB[:], lB[:])
                    RA = R_pool.tile([64, SQT], f32, tag="R")
                    RB = R_pool.tile([64, SQT], f32, tag="R")
                    nc.gpsimd.partition_broadcast(RA[:], recA[:])
                    nc.gpsimd.partition_broadcast(RB[:], recB[:])
                    yt = y_pool.tile([128, SQT], bf16, tag=f"y{i}")
                    nc.vector.tensor_mul(yt[0:64, :], oA[0:64, :], RA[:])
                    nc.vector.tensor_mul(yt[64:128, :], oB[0:64, :], RB[:])
                    y_strips[i] = yt
                y_hist[t] = y_strips

                # ==== output projection for s-tile t-1 (sw pipeline) ===
                if t > 0:
                    _emit_proj(t - 1)
                if t == n_st - 1:
                    _emit_proj(t)
    nc.compile()
    return nc


def _shard_inputs(x, W_attn, b_attn, seq=S, n_pairs=NPAIR, n_cores=8):
    import ml_dtypes

    bf = ml_dtypes.bfloat16
    dl = n_pairs * 2 * HD
    n_st, n_dc = seq // SQT, D // 128
    in_maps = []
    for c in range(n_cores):
        b, g = divmod(c, 2)
        qs = slice(dl * g, dl * g + dl)
        ks = slice(D + dl * g, D + dl * g + dl)
        # xT packed [t, dc, p, s]: x[b][SQT*t+s, 128*dc+p]
        xT = x[b].T.reshape(n_dc, 128, n_st, SQT).transpose(2, 0, 1, 3)
        wqk = np.concatenate([W_attn[:, qs], W_attn[:, ks]], axis=1)
        wv = W_attn[:, 2 * D + dl * g : 2 * D + dl * g + dl]
        in_maps.append({
            "xT": np.ascontiguousarray(xT).astype(bf),
            "wqk": np.ascontiguousarray(
                wqk.reshape(n_dc, 128, 2 * dl)
            ).astype(bf),
            "wv": np.ascontiguousarray(wv.reshape(n_dc, 128, dl)).astype(bf),
            "bqk": np.ascontiguousarray(
                np.concatenate([b_attn[qs], b_attn[ks]])
            ).astype(np.float32),
        })
    return in_maps


def kernel(x, W_attn, b_attn, W_proj, b_proj):
    import ml_dtypes
    from concourse.bass_utils import run_bass_kernel_spmd

    bf = ml_dtypes.bfloat16
    x = np.asarray(x, dtype=np.float32)
    W_attn = np.asarray(W_attn, dtype=np.float32)
    b_attn = np.asarray(b_attn, dtype=np.float32)
    W_proj = np.asarray(W_proj, dtype=np.float32)
    b_proj = np.asarray(b_proj, dtype=np.float32)

    if "nc" not in _CACHE:
        _CACHE["nc"] = _build()
    nc = _CACHE["nc"]

    in_maps = _shard_inputs(x, W_attn, b_attn)
    for c in range(8):
        g = c % 2
        in_maps[c]["wpj"] = np.ascontiguousarray(
            W_proj[DL * g : DL * g + DL, :]
        ).astype(bf)

    res = run_bass_kernel_spmd(nc, in_maps, core_ids=list(range(8)))

    # gather: out[b] = (partial_{2b} + partial_{2b+1})^T + b_v @ W_proj + b_proj
    bias_total = b_attn[2 * D :] @ W_proj + b_proj  # [D]
    out = np.empty((B, S, D), dtype=np.float32)
    for b in range(B):
        acc = res.results[2 * b]["outT"] + res.results[2 * b + 1]["outT"]
        # packed [t, oc, p, c] -> [s = 512t + c, d = 128 oc + p]
        out[b] = (
            acc.transpose(0, 3, 1, 2).reshape(S, D) + bias_total[None, :]
        )
    return out


# revision 23
# speedup vs baseline: 1.0329x; 1.0329x over previous
"""Causal self-attention (B=4, S=2048, D=1024, H=16) on 8 NeuronCores.

Sharding (per spec hint): data-parallel over B (4 batches) x tensor-parallel
over heads (2 groups of 8 heads) -> 8 shards, one per core. Each core:
  - QKV^T projection for its batch + head group (Megatron column split)
  - causal attention for its 8 heads, computed transposed: S^T[sk, sq] tiles
    so the softmax denominator (ones-column trick) and P@V both have the key
    dim on partitions
  - output projection with the Megatron row split of W_proj -> partial out^T
Host sums the two partials per batch and adds the bias terms.

Numerics: bf16 matmul operands (x, W, Q^T, K^T, V, P, y) with fp32 PSUM
accumulation; softmax (exp, denominators, reciprocal) in fp32. The softmax
max-subtraction is skipped: scores = QK/8 for these inputs are ~N(0,1), so
exp() cannot overflow and the result is mathematically identical.

Head pairs are packed: S^T for two heads runs as concurrent row-tiled
matmuls (K=64 each at tile_position (0,0)/(64,0)) into one 2-bank PSUM tile,
so exp covers both heads in ONE activation instruction (the per-instruction
~352-cycle ACT overhead is the attention bottleneck otherwise). Diagonal
chunks restrict the query-column range (c >= 128*d) so ~38% of the wasted
upper-triangle exp/matmul work is skipped; only the 128-wide boundary strip
needs the triangular mask.

b_attn's Q/K parts bias the scores nonlinearly -> added on-device (free,
during the PSUM->SBUF copies of Q^T/K^T). b_attn's V part and b_proj enter
the output linearly -> added host-side (y@W_proj + (b_v@W_proj + b_proj)).
"""

import numpy as np

B, S, D, H = 4, 2048, 1024, 16
HD = 64          # head dim
HG = H // 2      # heads per core (group)
NPAIR = HG // 2  # head pairs per core
DL = HG * HD     # local d (per group) = 512
SQT = 512        # query-tile width
SKC = 128        # key-chunk width

_CACHE = {}


def _build(seq=S, n_pairs=NPAIR):
    import concourse.bass as bass
    import concourse.tile as tile
    from concourse import bacc, mybir

    f32 = mybir.dt.float32
    bf16 = mybir.dt.bfloat16
    AF = mybir.ActivationFunctionType

    dl = n_pairs * 2 * HD            # local d
    n_st = seq // SQT                # s-tiles
    n_sk = seq // SKC                # sk chunks
    n_dc = D // 128                  # contraction chunks over D
    n_oc = D // 128                  # output-projection d_out chunks
    n_jc = 2 * n_pairs               # q + k row-chunks of 128
    n_sc = SQT // SKC                # sk chunks per s-tile

    nc = bacc.Bacc("TRN2", target_bir_lowering=False, debug=False)

    xT_d = nc.dram_tensor("xT", [n_st, n_dc, 128, SQT], bf16, kind="ExternalInput")
    wqk_d = nc.dram_tensor("wqk", [n_dc, 128, 2 * dl], bf16, kind="ExternalInput")
    wv_d = nc.dram_tensor("wv", [n_dc, 128, dl], bf16, kind="ExternalInput")
    wpj_d = nc.dram_tensor("wpj", [dl, D], bf16, kind="ExternalInput")
    bqk_d = nc.dram_tensor("bqk", [2 * dl], f32, kind="ExternalInput")
    outT_d = nc.dram_tensor(
        "outT", [n_st, n_oc, 128, SQT], f32, kind="ExternalOutput"
    )

    wpj_r = wpj_d[:, :].rearrange("(c p) o -> p c o", p=128)
    bqk_r = bqk_d.ap().rearrange("(c p) -> p c", p=128)

    with tile.TileContext(nc) as tc:
        with (
            tc.tile_pool(name="persist", bufs=1) as persist,
            tc.tile_pool(name="xt", bufs=3) as xt_pool,
            tc.tile_pool(name="qt", bufs=2) as qt_pool,
            tc.tile_pool(name="yp", bufs=2) as y_pool,
            tc.tile_pool(name="pp", bufs=10) as p_pool,
            tc.tile_pool(name="rp", bufs=4) as r_pool,
            tc.tile_pool(name="Rp", bufs=4) as R_pool,
            tc.tile_pool(name="so", bufs=6) as so_pool,
            tc.tile_pool(name="pm", bufs=2, space="PSUM") as pm,
            tc.tile_pool(name="st", bufs=2, space="PSUM") as st_pool,
            tc.tile_pool(name="pv", bufs=2, space="PSUM") as pv_pool,
        ):
            # ---- persistent loads (split per d-chunk for early start) --
            wqk_sb = persist.tile([128, n_dc, 2 * dl], bf16, tag="wqk")
            wv_sb = persist.tile([128, n_dc, dl], bf16, tag="wv")
            wpj_sb = persist.tile([128, n_pairs, D], bf16, tag="wpj")
            bqk_sb = persist.tile([128, n_jc], f32, tag="bqk")
            nc.sync.dma_start(bqk_sb[:], bqk_r)

            kt = persist.tile([128, n_pairs, seq], bf16, tag="kt")
            v = persist.tile([128, n_sk, 2 * n_pairs, HD + 1], bf16, tag="v")

            # triangular boundary mask: M[p, c] = 1 iff c >= p
            mask0 = persist.tile([128, SKC], bf16, tag="mask0")
            nc.gpsimd.memset(mask0[:], 1.0)
            nc.gpsimd.affine_select(
                out=mask0[:], in_=mask0[:],
                compare_op=mybir.AluOpType.is_ge,
                fill=0.0, base=0,
                pattern=[[1, SKC]], channel_multiplier=-1,
            )

            y_hist = {}

            def _emit_proj(tp):
                qp = SQT * tp
                strips = y_hist.pop(tp)
                for oc in range(n_oc):
                    ps = pm.tile([128, SQT], f32, tag="pm")
                    for i in range(n_pairs):
                        nc.tensor.matmul(
                            ps[:],
                            wpj_sb[:, i, 128 * oc : 128 * oc + 128],
                            strips[i][:],
                            start=(i == 0), stop=(i == n_pairs - 1),
                        )
                    ot = so_pool.tile([128, SQT], f32, tag="ot")
                    nc.vector.tensor_copy(ot[:], ps[:])
                    nc.sync.dma_start(outT_d[tp, oc, :, :], ot[:])

            for t in range(n_st):
                q0 = SQT * t
                # ==== QKV^T projection for s-tile t ====================
                xt = xt_pool.tile([128, n_dc, SQT], bf16, tag="xt")
                if t == 0:
                    # interleave so MM(dc) inputs arrive in accumulation order
                    for dc in range(n_dc):
                        nc.sync.dma_start(xt[:, dc, :], xT_d[t, dc, :, :])
                        nc.sync.dma_start(wqk_sb[:, dc, :], wqk_d[dc, :, :])
                    for dc in range(n_dc):
                        nc.sync.dma_start(wv_sb[:, dc, :], wv_d[dc, :, :])
                    for i in range(n_pairs):
                        nc.sync.dma_start(wpj_sb[:, i, :], wpj_r[:, i, :])
                else:
                    for dc in range(n_dc):
                        nc.sync.dma_start(xt[:, dc, :], xT_d[t, dc, :, :])
                qt = qt_pool.tile([128, n_pairs, SQT], bf16, tag="qt")
                for jc in range(n_jc):
                    ps = pm.tile([128, SQT], f32, tag="pm")
                    for dc in range(n_dc):
                        nc.tensor.matmul(
                            ps[:],
                            wqk_sb[:, dc, 128 * jc : 128 * jc + 128],
                            xt[:, dc, :],
                            start=(dc == 0), stop=(dc == n_dc - 1),
                        )
                    dst = (
                        qt[:, jc, :] if jc < n_pairs
                        else kt[:, jc - n_pairs, q0 : q0 + SQT]
                    )
                    nc.vector.tensor_scalar_add(
                        dst, ps[:], bqk_sb[:, jc : jc + 1]
                    )
                for sc in range(n_sc):
                    k = n_sc * t + sc
                    ps = pm.tile([128, dl], f32, tag="pm")
                    for dc in range(n_dc):
                        nc.tensor.matmul(
                            ps[:],
                            xt[:, dc, 128 * sc : 128 * sc + 128],
                            wv_sb[:, dc, :],
                            start=(dc == 0), stop=(dc == n_dc - 1),
                        )
                    nc.vector.tensor_copy(
                        v[:, k, :, 0:HD],
                        ps[:].rearrange("p (h c) -> p h c", h=2 * n_pairs),
                    )
                    nc.vector.memset(v[:, k, :, HD : HD + 1], 1.0)

                # ==== causal attention for sq-tile t ===================
                nch = n_sc * (t + 1)  # sk chunks 0..nch-1
                y_strips = {}
                for i in range(n_pairs):
                    oA = pv_pool.tile([HD + 1, SQT], f32, tag="pv")
                    oB = pv_pool.tile([HD + 1, SQT], f32, tag="pv")
                    for k in range(nch):
                        dgi = k - n_sc * t   # >=0: diagonal chunk index
                        c0 = max(0, 128 * dgi)  # first valid query column
                        stk = st_pool.tile([128, 2, SQT], f32, tag="st")
                        nc.tensor.matmul(
                            stk[:, 0, c0:SQT],
                            kt[0:64, i, 128 * k : 128 * k + 128],
                            qt[0:64, i, c0:SQT],
                            start=True, stop=True, tile_position=(0, 0),
                        )
                        nc.tensor.matmul(
                            stk[:, 1, c0:SQT],
                            kt[64:128, i, 128 * k : 128 * k + 128],
                            qt[64:128, i, c0:SQT],
                            start=True, stop=True, tile_position=(64, 0),
                        )
                        pk = p_pool.tile([128, 2, SQT], bf16, tag="p")
                        nc.scalar.activation(
                            pk[:, :, c0:SQT], stk[:, :, c0:SQT],
                            AF.Exp, scale=0.125,
                        )
                        if dgi >= 0:
                            # triangular boundary strip [c0, c0+128)
                            for h in range(2):
                                nc.vector.tensor_mul(
                                    pk[:, h, c0 : c0 + SKC],
                                    pk[:, h, c0 : c0 + SKC],
                                    mask0[:],
                                )
                        for h, o in ((0, oA), (1, oB)):
                            nc.tensor.matmul(
                                o[:, c0:SQT],
                                v[:, k, 2 * i + h, :],
                                pk[:, h, c0:SQT],
                                start=(k == 0), stop=(k == nch - 1),
                                skip_group_check=True,
                            )
                    # copy l rows PSUM->SBUF (the approx-recip bit trick is
                    # PSUM-incompatible), then fast reciprocal on SBUF
                    lA = r_pool.tile([1, SQT], f32, tag="lc")
                    lB = r_pool.tile([1, SQT], f32, tag="lc")
                    nc.vector.tensor_copy(lA[:], oA[HD : HD + 1, :])
                    nc.vector.tensor_copy(lB[:], oB[HD : HD + 1, :])
                    recA = r_pool.tile([1, SQT], f32, tag="rec")
                    recB = r_pool.tile([1, SQT], f32, tag="rec")
                    nc.vector.reciprocal_approx_fast(recA[:], lA[:])
                    nc.vector.reciprocal_approx_fast(recB[:], lB[:])
                    RA = R_pool.tile([64, SQT], f32, tag="R")
                    RB = R_pool.tile([64, SQT], f32, tag="R")
                    nc.gpsimd.partition_broadcast(RA[:], recA[:])
                    nc.gpsimd.partition_broadcast(RB[:], recB[:])
                    yt = y_pool.tile([128, SQT], bf16, tag=f"y{i}")
                    nc.vector.tensor_mul(yt[0:64, :], oA[0:64, :], RA[:])
                    nc.vector.tensor_mul(yt[64:128, :], oB[0:64, :], RB[:])
                    y_strips[i] = yt
                y_hist[t] = y_strips

                # ==== output projection for s-tile t-1 (sw pipeline) ===
                if t > 0:
                    _emit_proj(t - 1)
                if t == n_st - 1:
                    _emit_proj(t)
    nc.compile()
    return nc


def _shard_inputs(x, W_attn, b_attn, seq=S, n_pairs=NPAIR, n_cores=8):
    import ml_dtypes

    bf = ml_dtypes.bfloat16
    dl = n_pairs * 2 * HD
    n_st, n_dc = seq // SQT, D // 128
    in_maps = []
    for c in range(n_cores):
        b, g = divmod(c, 2)
        qs = slice(dl * g, dl * g + dl)
        ks = slice(D + dl * g, D + dl * g + dl)
        # xT packed [t, dc, p, s]: x[b][SQT*t+s, 128*dc+p]
        xT = x[b].T.reshape(n_dc, 128, n_st, SQT).transpose(2, 0, 1, 3)
        wqk = np.concatenate([W_attn[:, qs], W_attn[:, ks]], axis=1)
        wv = W_attn[:, 2 * D + dl * g : 2 * D + dl * g + dl]
        in_maps.append({
            "xT": np.ascontiguousarray(xT).astype(bf),
            "wqk": np.ascontiguousarray(
                wqk.reshape(n_dc, 128, 2 * dl)
            ).astype(bf),
            "wv": np.ascontiguousarray(wv.reshape(n_dc, 128, dl)).astype(bf),
            "bqk": np.ascontiguousarray(
                np.concatenate([b_attn[qs], b_attn[ks]])
            ).astype(np.float32),
        })
    return in_maps


def kernel(x, W_attn, b_attn, W_proj, b_proj):
    import ml_dtypes
    from concourse.bass_utils import run_bass_kernel_spmd

    bf = ml_dtypes.bfloat16
    x = np.asarray(x, dtype=np.float32)
    W_attn = np.asarray(W_attn, dtype=np.float32)
    b_attn = np.asarray(b_attn, dtype=np.float32)
    W_proj = np.asarray(W_proj, dtype=np.float32)
    b_proj = np.asarray(b_proj, dtype=np.float32)

    if "nc" not in _CACHE:
        _CACHE["nc"] = _build()
    nc = _CACHE["nc"]

    in_maps = _shard_inputs(x, W_attn, b_attn)
    for c in range(8):
        g = c % 2
        in_maps[c]["wpj"] = np.ascontiguousarray(
            W_proj[DL * g : DL * g + DL, :]
        ).astype(bf)

    res = run_bass_kernel_spmd(nc, in_maps, core_ids=list(range(8)))

    # gather: out[b] = (partial_{2b} + partial_{2b+1})^T + b_v @ W_proj + b_proj
    bias_total = b_attn[2 * D :] @ W_proj + b_proj  # [D]
    out = np.empty((B, S, D), dtype=np.float32)
    for b in range(B):
        acc = res.results[2 * b]["outT"] + res.results[2 * b + 1]["outT"]
        # packed [t, oc, p, c] -> [s = 512t + c, d = 128 oc + p]
        out[b] = (
            acc.transpose(0, 3, 1, 2).reshape(S, D) + bias_total[None, :]
        )
    return out
